# revision 7
# baseline (speedup 1.0000x reference)
"""ATG-GRU (t-product graph GRU) Trainium2 kernel.

Sharding: data-parallel over batch B=32 across 8 cores (B_local=4).
Math: real-domain block-circulant formulation of the t-product gconv:
  gconv(X)[b,n,h,r] = sum_{m,f,r1+r2+r3=r (mod R)} adj[n,m,r1] X[b,m,f,r2] W[f,h,r3]
Two matmul stages per gate group:
  FT  (contract (r',f)):  lhsT = activation in "A" layout [(r',f), (b,n)],
                          rhs = host-built circulant weight Wc [(r',f),(g,r,h)]
                          -> PSUM [n, (g,r,h)] per (b, n-chunk)  ("B" layout)
  Agg (contract m, circular shift over r): lhsT = adjT_q [m, n'], rhs = pre-acts
      in B layout; accumulate over q with r-block rotation of the free axis;
      bias added via rank-1 matmul (ones row x bias row).
All matmuls full fp32 (the recurrence amplifies rounding ~2e3x; fp16/fp32r operands lose ~all precision).
"""
import sys

sys.path.insert(0, "/opt/trn_rl_repo")

import numpy as np

import concourse.bass as bass
import concourse.mybir as mybir
import concourse.tile as tile
from concourse import bacc
from concourse.bass_utils import run_bass_kernel_spmd
from concourse.masks import make_identity

B, T, N, F, H, R, E = 32, 8, 150, 16, 32, 4, 16
NCORES = 8
BL = B // NCORES          # 4 local batch
NCH = [(0, 128), (128, N - 128)]   # node chunks (offset, size)
F16 = mybir.dt.float16
F32 = mybir.dt.float32

_cache = {}


def host_prep(inputs, U, Wxz0, Wxr0, Wxh0, Whz0, Whr0, B0,
              Wxz1, Wxr1, Wxh1, Whz1, Whr1, B1):
    """Host-side layout transforms + the tiny adjacency computation."""
    Uf = np.fft.fft(U.astype(np.float64), axis=-1)
    A = np.fft.ifft(np.einsum('ner,mer->nmr', Uf, Uf), axis=-1).real
    A = np.maximum(A, 0.0)
    A = A - A.max(axis=1, keepdims=True)
    Ea = np.exp(A)
    adj = (Ea / Ea.sum(axis=1, keepdims=True)).astype(np.float32)  # [N,N,R]
    adjt = np.ascontiguousarray(adj.transpose(2, 1, 0))  # [q, m, n'] = adj[n',m,q]

    def circ(W):
        """W [D,H,R] -> Wc [(r'*D+f), (r*H+h)] = W[f,h,(r-r')%R]."""
        D = W.shape[0]
        Wc = np.zeros((R * D, R * H), np.float32)
        for rp in range(R):
            for r in range(R):
                Wc[rp * D:(rp + 1) * D, r * H:(r + 1) * H] = W[:, :, (r - rp) % R]
        return Wc

    wcx = [np.concatenate([circ(Wxz0), circ(Wxr0), circ(Wxh0)], axis=1),
           np.concatenate([circ(Wxz1), circ(Wxr1), circ(Wxh1)], axis=1)]
    wch = [np.concatenate([circ(Whz0), circ(Whr0)], axis=1),
           np.concatenate([circ(Whz1), circ(Whr1)], axis=1)]
    wcr = [circ(Whr0), circ(Whr1)]
    # bias rows: brow[(l,g), (b, ro, h)] = B_l[g, h, ro]
    brow = np.zeros((6, BL * R * H), np.float32)
    for l, Bb in ((0, B0), (1, B1)):
        for g in range(3):
            brow[l * 3 + g] = np.tile(Bb[g].T.reshape(-1), BL)
    brow = brow.reshape(1, 6 * BL * R * H)
    xs = []
    for c in range(NCORES):
        xl = inputs[c * BL:(c + 1) * BL]                    # [BL,T,N,F,R]
        xs.append(np.ascontiguousarray(
            xl.transpose(1, 4, 3, 0, 2).reshape(T, R * F, BL * N)).astype(np.float32))
    return adjt, wcx, wch, wcr, brow, xs


def _emit(nc):
    dp = nc.declare_dram_parameter
    xin = dp("xin", [T, R * F, BL * N], F32, isOutput=False)
    adjt = dp("adjt", [R, N, N], F32, isOutput=False)
    wcx0 = dp("wcx0", [R * F, 3 * R * H], F32, isOutput=False)
    wcx1 = dp("wcx1", [R * H, 3 * R * H], F32, isOutput=False)
    wch0 = dp("wch0", [R * H, 2 * R * H], F32, isOutput=False)
    wch1 = dp("wch1", [R * H, 2 * R * H], F32, isOutput=False)
    wcr0 = dp("wcr0", [R * H, R * H], F32, isOutput=False)
    wcr1 = dp("wcr1", [R * H, R * H], F32, isOutput=False)
    brow = dp("brow", [1, 6 * BL * R * H], F32, isOutput=False)
    outp = dp("o", [T, N, BL * R * H], F32, isOutput=True)
    hlast = dp("hl", [2, N, BL * R * H], F32, isOutput=True)

    SIG = mybir.ActivationFunctionType.Sigmoid
    TANH = mybir.ActivationFunctionType.Tanh

    with tile.TileContext(nc) as tc:
        with tc.tile_pool(name="const", bufs=1) as cpool, \
             tc.tile_pool(name="xa", bufs=1) as xpool, \
             tc.tile_pool(name="ha", bufs=5) as hapool, \
             tc.tile_pool(name="hnb", bufs=10) as hnbpool, \
             tc.tile_pool(name="gat", bufs=4) as gpool, \
             tc.tile_pool(name="work", bufs=4) as wpool, \
             tc.tile_pool(name="pre", bufs=3) as ppool, \
             tc.tile_pool(name="ftps", bufs=2, space="PSUM") as ftps, \
             tc.tile_pool(name="aggps", bufs=2, space="PSUM") as aggps:

            # ---- constants ----
            wx = [cpool.tile([R * F, 3, R, H], F32, tag="wcx0", name="wcx0"),
                  cpool.tile([R * H, 3, R, H], F32, tag="wcx1", name="wcx1")]
            nc.sync.dma_start(out=wx[0][:], in_=wcx0.ap().rearrange("k (g r h) -> k g r h", g=3, r=R))
            nc.sync.dma_start(out=wx[1][:], in_=wcx1.ap().rearrange("k (g r h) -> k g r h", g=3, r=R))
            wh = [cpool.tile([R * H, 2, R, H], F32, tag="wch0", name="wch0"),
                  cpool.tile([R * H, 2, R, H], F32, tag="wch1", name="wch1")]
            nc.sync.dma_start(out=wh[0][:], in_=wch0.ap().rearrange("k (g r h) -> k g r h", g=2, r=R))
            nc.sync.dma_start(out=wh[1][:], in_=wch1.ap().rearrange("k (g r h) -> k g r h", g=2, r=R))
            wr = [cpool.tile([R * H, R, H], F32, tag="wcr0", name="wcr0"),
                  cpool.tile([R * H, R, H], F32, tag="wcr1", name="wcr1")]
            nc.sync.dma_start(out=wr[0][:], in_=wcr0.ap().rearrange("k (r h) -> k r h", r=R))
            nc.sync.dma_start(out=wr[1][:], in_=wcr1.ap().rearrange("k (r h) -> k r h", r=R))
            adjts = []
            for q in range(R):
                row = []
                for (mo, ms) in NCH:
                    t_ = cpool.tile([128, N], F32, tag=f"adjt{q}{mo}", name=f"adjt{q}{mo}")
                    nc.sync.dma_start(out=t_[:ms], in_=adjt[q, mo:mo + ms, :])
                    row.append(t_)
                adjts.append(row)
            btile = cpool.tile([1, 6, BL, R, H], F32, tag="brow", name="brow")
            nc.sync.dma_start(out=btile[:], in_=brow.ap().rearrange(
                "o (s b r h) -> o s b r h", s=6, b=BL, r=R))
            ones = cpool.tile([1, N], F32, tag="ones", name="ones")
            nc.vector.memset(ones[:], 1.0)
            ident = cpool.tile([128, 128], F32, tag="ident", name="ident")
            make_identity(nc, ident[:])
            xa = []
            for t_ in range(T):
                xt = xpool.tile([R * F, BL, N], F32, tag=f"x{t_}", name=f"x{t_}")
                nc.sync.dma_start(out=xt[:], in_=xin[t_].rearrange("k (b n) -> k b n", b=BL))
                xa.append(xt)

            def transpose_to_a(src_b, dtile):
                """src_b: 2 B-layout fp32 tiles [ns, BL, R, H] -> dtile fp32 [128, BL, N]."""
                for ic, (no, ns) in enumerate(NCH):
                    pt = ftps.tile([128, BL, 128], F32, tag="ftp", name="trp4")
                    for b in range(BL):
                        src = src_b[ic][:ns, b].rearrange("p r h -> p (r h)")
                        nc.tensor.transpose(pt[:, b, :ns], src, ident[:ns, :ns])
                    for b in range(BL):
                        nc.any.tensor_copy(out=dtile[:, b, no:no + ns], in_=pt[:, b, :ns])

            def agg(lyr, gidx, pre, gsel, psum_tag):
                """Aggregate one gate: returns list of 2 PSUM tiles [ns, BL, R, H].
                pre: 2 tiles [ms, BL, (G,) R, H]; gsel: gate index or None."""
                out = []
                for ic, (no, ns) in enumerate(NCH):
                    pg = aggps.tile([128, BL, R, H], F32, tag=psum_tag)
                    nc.tensor.matmul(pg[:ns], ones[:, no:no + ns],
                                     btile[0:1, lyr * 3 + gidx].rearrange("o b r h -> o (b r h)"),
                                     start=True, stop=False)
                    for q in range(R):
                        for im, (mo, ms) in enumerate(NCH):
                            last = (q == R - 1) and (im == len(NCH) - 1)
                            lt = adjts[q][im][:ms, no:no + ns]
                            src = pre[im][:ms] if gsel is None else pre[im][:ms, :, gsel]
                            if q == 0:
                                nc.tensor.matmul(pg[:ns], lt, src,
                                                 start=False, stop=last)
                            else:
                                nc.tensor.matmul(pg[:ns, :, q:R], lt, src[:, :, 0:R - q],
                                                 start=False, stop=False)
                                nc.tensor.matmul(pg[:ns, :, 0:q], lt, src[:, :, R - q:R],
                                                 start=False, stop=last)
                    out.append(pg)
                return out

            def cell(lyr, x_a, h_a, h_b, t_):
                """One TensorGRUCell; returns (hnew_a, hnew_b)."""
                # --- FT z,r gates ---
                pzr = [ppool.tile([128, BL, 2, R, H], F32, tag="pzr0", name="pzr0"),
                       ppool.tile([128, BL, 2, R, H], F32, tag="pzr1", name="pzr1")]
                for ic, (no, ns) in enumerate(NCH):
                    for b in range(BL):
                        ps1 = ftps.tile([128, 2, R, H], F32, tag="ftp", name="ftzr")
                        nc.tensor.matmul(ps1[:ns], x_a[:, b, no:no + ns],
                                         wx[lyr][:, 0:2], start=True, stop=h_a is None)
                        if h_a is not None:
                            nc.tensor.matmul(ps1[:ns], h_a[:, b, no:no + ns],
                                             wh[lyr][:], start=False, stop=True)
                        nc.any.tensor_copy(out=pzr[ic][:ns, b], in_=ps1[:ns])

                # --- Agg z, r + sigmoid ---
                pz = agg(lyr, 0, pzr, 0, "az")
                pr = agg(lyr, 1, pzr, 1, "ar")
                zb, rb = [], []
                for ic, (no, ns) in enumerate(NCH):
                    zt = gpool.tile([128, BL, R, H], F32, tag="zb", name="zb")
                    rt = gpool.tile([128, BL, R, H], F32, tag="rb", name="rb")
                    nc.scalar.activation(out=zt[:ns], in_=pz[ic][:ns], func=SIG, scale=1.0)
                    nc.scalar.activation(out=rt[:ns], in_=pr[ic][:ns], func=SIG, scale=1.0)
                    zb.append(zt)
                    rb.append(rt)

                # --- Rg*Hs -> A layout ---
                rha = None
                if h_b is not None:
                    rhs_b = []
                    for ic, (no, ns) in enumerate(NCH):
                        rh = wpool.tile([128, BL, R, H], F32, tag="rghs", name="rghs")
                        eng = nc.vector if ic == 0 else nc.gpsimd
                        eng.tensor_tensor(rh[:ns], rb[ic][:ns], h_b[ic][:ns],
                                          mybir.AluOpType.mult)
                        rhs_b.append(rh)
                    rha = wpool.tile([R * H, BL, N], F32, tag="rha", name="rha")
                    transpose_to_a(rhs_b, rha)

                # --- FT h gate (X part + RgHs part in one PSUM) ---
                ph = [ppool.tile([128, BL, R, H], F32, tag="ph0", name="ph0"),
                      ppool.tile([128, BL, R, H], F32, tag="ph1", name="ph1")]
                for ic, (no, ns) in enumerate(NCH):
                    for b in range(BL):
                        ps2 = ftps.tile([128, R, H], F32, tag="ftp", name="fth")
                        nc.tensor.matmul(ps2[:ns], x_a[:, b, no:no + ns],
                                         wx[lyr][:, 2], start=True, stop=rha is None)
                        if rha is not None:
                            nc.tensor.matmul(ps2[:ns], rha[:, b, no:no + ns],
                                             wr[lyr][:], start=False, stop=True)
                        nc.any.tensor_copy(out=ph[ic][:ns, b], in_=ps2[:ns])

                # --- Agg h + tanh ---
                pht = agg(lyr, 2, ph, None, "ah")
                htb = []
                for ic, (no, ns) in enumerate(NCH):
                    ht = gpool.tile([128, BL, R, H], F32, tag="htb", name="htb")
                    nc.scalar.activation(out=ht[:ns], in_=pht[ic][:ns], func=TANH, scale=1.0)
                    htb.append(ht)

                # --- mix: Hnew = Ht + Z*(Hs - Ht) ; t=0: Hnew = Ht - Z*Ht ---
                hnb = []
                for ic, (no, ns) in enumerate(NCH):
                    hn = hnbpool.tile([128, BL, R, H], F32, tag="hnb", name="hnb")
                    if h_b is None:
                        d = wpool.tile([128, BL, R, H], F32, tag="mixd", name="mixd")
                        nc.vector.tensor_tensor(d[:ns], zb[ic][:ns], htb[ic][:ns],
                                                mybir.AluOpType.mult)
                        nc.vector.tensor_tensor(hn[:ns], htb[ic][:ns], d[:ns],
                                                mybir.AluOpType.subtract)
                    else:
                        d = wpool.tile([128, BL, R, H], F32, tag="mixd", name="mixd")
                        nc.vector.tensor_tensor(d[:ns], h_b[ic][:ns], htb[ic][:ns],
                                                mybir.AluOpType.subtract)
                        eng = nc.gpsimd if ic == 0 else nc.vector
                        eng.tensor_tensor(d[:ns], d[:ns], zb[ic][:ns],
                                          mybir.AluOpType.mult)
                        nc.vector.tensor_tensor(hn[:ns], htb[ic][:ns], d[:ns],
                                                mybir.AluOpType.add)
                    hnb.append(hn)
                hna = hapool.tile([R * H, BL, N], F32, tag="hna", name="hna")
                transpose_to_a(hnb, hna)
                return hna, hnb

            # ---- recurrence ----
            hA = [None, None]
            hB = [None, None]
            for t_ in range(T):
                x1a = hA[0]  # layer1 consumes previous-step o0 for t>=1
                hA[0], hB[0] = cell(0, xa[t_], hA[0], hB[0], t_)
                if t_ == 0:
                    x1a = hA[0]
                hA[1], hB[1] = cell(1, x1a, hA[1], hB[1], t_)
                for ic, (no, ns) in enumerate(NCH):
                    nc.sync.dma_start(
                        out=outp[t_, no:no + ns, :],
                        in_=hB[1][ic][:ns].rearrange("p b r h -> p (b r h)"))
            for l in range(2):
                for ic, (no, ns) in enumerate(NCH):
                    nc.sync.dma_start(
                        out=hlast[l, no:no + ns, :],
                        in_=hB[l][ic][:ns].rearrange("p b r h -> p (b r h)"))
    nc.compile()
    return nc


def _get_nc():
    if "nc" not in _cache:
        nc = bacc.Bacc("TRN2", target_bir_lowering=False, debug=False,
                       num_devices=NCORES)
        _cache["nc"] = _emit(nc)
    return _cache["nc"]


def kernel(inputs, U, Wxz0, Wxr0, Wxh0, Whz0, Whr0, B0,
           Wxz1, Wxr1, Wxh1, Whz1, Whr1, B1):
    args = [np.asarray(a, np.float32) for a in
            (inputs, U, Wxz0, Wxr0, Wxh0, Whz0, Whr0, B0,
             Wxz1, Wxr1, Wxh1, Whz1, Whr1, B1)]
    adjt, wcx, wch, wcr, brow, xs = host_prep(*args)
    nc = _get_nc()
    shared = dict(adjt=adjt, wcx0=wcx[0], wcx1=wcx[1], wch0=wch[0],
                  wch1=wch[1], wcr0=wcr[0], wcr1=wcr[1], brow=brow)
    in_maps = [dict(shared, xin=xs[c]) for c in range(NCORES)]
    res = run_bass_kernel_spmd(nc, in_maps, list(range(NCORES)),
                               **_cache.get("run_kwargs", {}))
    _cache["last_results"] = res
    outs, hls = [], []
    for c in range(NCORES):
        o = res.results[c]["o"].reshape(T, N, BL, R, H)
        outs.append(np.ascontiguousarray(o.transpose(2, 0, 1, 4, 3)))
        hl = res.results[c]["hl"].reshape(2, N, BL, R, H)
        hls.append(np.ascontiguousarray(hl.transpose(0, 2, 1, 4, 3)))
    out = np.concatenate(outs, axis=0).astype(np.float32)
    h_last = np.concatenate(hls, axis=1).astype(np.float32)
    return out, h_last


# revision 9
# speedup vs baseline: 1.0964x; 1.0964x over previous
"""ATG-GRU (t-product graph GRU) Trainium2 kernel.

Sharding: data-parallel over batch B=32 across 8 cores (B_local=4).
Math: real-domain block-circulant formulation of the t-product gconv:
  gconv(X)[b,n,h,r] = sum_{m,f,r1+r2+r3=r (mod R)} adj[n,m,r1] X[b,m,f,r2] W[f,h,r3]
Two matmul stages per gate group:
  FT  (contract (r',f)):  lhsT = activation in "A" layout [(r',f), (b,n)],
                          rhs = host-built circulant weight Wc [(r',f),(g,r,h)]
                          -> PSUM [n, (g,r,h)] per (b, n-chunk)  ("B" layout)
  Agg (contract m, circular shift over r): lhsT = adjT_q [m, n'], rhs = pre-acts
      in B layout; accumulate over q with r-block rotation of the free axis;
      bias added via rank-1 matmul (ones row x bias row).
All matmuls full fp32 (the recurrence amplifies rounding ~2e3x; fp16/fp32r operands lose ~all precision).
"""
import sys

sys.path.insert(0, "/opt/trn_rl_repo")

import numpy as np

import concourse.bass as bass
import concourse.mybir as mybir
import concourse.tile as tile
from concourse import bacc
from concourse.bass_utils import run_bass_kernel_spmd
from concourse.masks import make_identity

B, T, N, F, H, R, E = 32, 8, 150, 16, 32, 4, 16
NCORES = 8
BL = B // NCORES          # 4 local batch
NCH = [(0, 128), (128, N - 128)]   # node chunks (offset, size)
F16 = mybir.dt.float16
F32 = mybir.dt.float32

_cache = {}


def host_prep(inputs, U, Wxz0, Wxr0, Wxh0, Whz0, Whr0, B0,
              Wxz1, Wxr1, Wxh1, Whz1, Whr1, B1):
    """Host-side layout transforms + the tiny adjacency computation."""
    Uf = np.fft.fft(U.astype(np.float64), axis=-1)
    A = np.fft.ifft(np.einsum('ner,mer->nmr', Uf, Uf), axis=-1).real
    A = np.maximum(A, 0.0)
    A = A - A.max(axis=1, keepdims=True)
    Ea = np.exp(A)
    adj = (Ea / Ea.sum(axis=1, keepdims=True)).astype(np.float32)  # [N,N,R]
    adjt = np.ascontiguousarray(adj.transpose(2, 1, 0))  # [q, m, n'] = adj[n',m,q]

    def circ(W):
        """W [D,H,R] -> Wc [(r'*D+f), (r*H+h)] = W[f,h,(r-r')%R]."""
        D = W.shape[0]
        Wc = np.zeros((R * D, R * H), np.float32)
        for rp in range(R):
            for r in range(R):
                Wc[rp * D:(rp + 1) * D, r * H:(r + 1) * H] = W[:, :, (r - rp) % R]
        return Wc

    wcx = [np.concatenate([circ(Wxz0), circ(Wxr0), circ(Wxh0)], axis=1),
           np.concatenate([circ(Wxz1), circ(Wxr1), circ(Wxh1)], axis=1)]
    wch = [np.concatenate([circ(Whz0), circ(Whr0)], axis=1),
           np.concatenate([circ(Whz1), circ(Whr1)], axis=1)]
    wcr = [circ(Whr0), circ(Whr1)]
    # bias rows: brow[(l,g), (b, ro, h)] = B_l[g, h, ro]
    brow = np.zeros((6, BL * R * H), np.float32)
    for l, Bb in ((0, B0), (1, B1)):
        for g in range(3):
            brow[l * 3 + g] = np.tile(Bb[g].T.reshape(-1), BL)
    brow = brow.reshape(1, 6 * BL * R * H)
    xs = []
    for c in range(NCORES):
        xl = inputs[c * BL:(c + 1) * BL]                    # [BL,T,N,F,R]
        xs.append(np.ascontiguousarray(
            xl.transpose(1, 4, 3, 0, 2).reshape(T, R * F, BL * N)).astype(np.float32))
    return adjt, wcx, wch, wcr, brow, xs


def _emit(nc):
    dp = nc.declare_dram_parameter
    xin = dp("xin", [T, R * F, BL * N], F32, isOutput=False)
    adjt = dp("adjt", [R, N, N], F32, isOutput=False)
    wcx0 = dp("wcx0", [R * F, 3 * R * H], F32, isOutput=False)
    wcx1 = dp("wcx1", [R * H, 3 * R * H], F32, isOutput=False)
    wch0 = dp("wch0", [R * H, 2 * R * H], F32, isOutput=False)
    wch1 = dp("wch1", [R * H, 2 * R * H], F32, isOutput=False)
    wcr0 = dp("wcr0", [R * H, R * H], F32, isOutput=False)
    wcr1 = dp("wcr1", [R * H, R * H], F32, isOutput=False)
    brow = dp("brow", [1, 6 * BL * R * H], F32, isOutput=False)
    outp = dp("o", [T, N, BL * R * H], F32, isOutput=True)
    hlast = dp("hl", [2, N, BL * R * H], F32, isOutput=True)

    SIG = mybir.ActivationFunctionType.Sigmoid
    TANH = mybir.ActivationFunctionType.Tanh

    with tile.TileContext(nc) as tc:
        with tc.tile_pool(name="const", bufs=1) as cpool, \
             tc.tile_pool(name="xa", bufs=1) as xpool, \
             tc.tile_pool(name="ha", bufs=5) as hapool, \
             tc.tile_pool(name="hnb", bufs=10) as hnbpool, \
             tc.tile_pool(name="gat", bufs=4) as gpool, \
             tc.tile_pool(name="work", bufs=4) as wpool, \
             tc.tile_pool(name="pre", bufs=3) as ppool, \
             tc.tile_pool(name="ftps", bufs=2, space="PSUM") as ftps, \
             tc.tile_pool(name="aggps", bufs=3, space="PSUM") as aggps, \
             tc.tile_pool(name="trps", bufs=1, space="PSUM") as trps:

            # ---- constants ----
            wx = [cpool.tile([R * F, 3, R, H], F32, tag="wcx0", name="wcx0"),
                  cpool.tile([R * H, 3, R, H], F32, tag="wcx1", name="wcx1")]
            nc.sync.dma_start(out=wx[0][:], in_=wcx0.ap().rearrange("k (g r h) -> k g r h", g=3, r=R))
            nc.sync.dma_start(out=wx[1][:], in_=wcx1.ap().rearrange("k (g r h) -> k g r h", g=3, r=R))
            wh = [cpool.tile([R * H, 2, R, H], F32, tag="wch0", name="wch0"),
                  cpool.tile([R * H, 2, R, H], F32, tag="wch1", name="wch1")]
            nc.sync.dma_start(out=wh[0][:], in_=wch0.ap().rearrange("k (g r h) -> k g r h", g=2, r=R))
            nc.sync.dma_start(out=wh[1][:], in_=wch1.ap().rearrange("k (g r h) -> k g r h", g=2, r=R))
            wr = [cpool.tile([R * H, R, H], F32, tag="wcr0", name="wcr0"),
                  cpool.tile([R * H, R, H], F32, tag="wcr1", name="wcr1")]
            nc.sync.dma_start(out=wr[0][:], in_=wcr0.ap().rearrange("k (r h) -> k r h", r=R))
            nc.sync.dma_start(out=wr[1][:], in_=wcr1.ap().rearrange("k (r h) -> k r h", r=R))
            adjts = []
            for q in range(R):
                row = []
                for (mo, ms) in NCH:
                    t_ = cpool.tile([128, N], F32, tag=f"adjt{q}{mo}", name=f"adjt{q}{mo}")
                    nc.sync.dma_start(out=t_[:ms], in_=adjt[q, mo:mo + ms, :])
                    row.append(t_)
                adjts.append(row)
            btile = cpool.tile([1, 6, BL, R, H], F32, tag="brow", name="brow")
            nc.sync.dma_start(out=btile[:], in_=brow.ap().rearrange(
                "o (s b r h) -> o s b r h", s=6, b=BL, r=R))
            ones = cpool.tile([1, N], F32, tag="ones", name="ones")
            nc.vector.memset(ones[:], 1.0)
            ident = cpool.tile([128, 128], F32, tag="ident", name="ident")
            make_identity(nc, ident[:])
            xa = []
            for t_ in range(T):
                xt = xpool.tile([R * F, BL, N], F32, tag=f"x{t_}", name=f"x{t_}")
                nc.sync.dma_start(out=xt[:], in_=xin[t_].rearrange("k (b n) -> k b n", b=BL))
                xa.append(xt)

            def transpose_to_a(src_b, dtile):
                """src_b: 2 B-layout fp32 tiles [ns, BL, R, H] -> dtile fp32 [128, BL, N]."""
                for b in range(BL):
                    for ic, (no, ns) in enumerate(NCH):
                        pt = trps.tile([128, 128], F32, tag="trp", name="trp")
                        src = src_b[ic][:ns, b].rearrange("p r h -> p (r h)")
                        nc.tensor.transpose(pt[:, :ns], src, ident[:ns, :ns])
                        nc.any.tensor_copy(out=dtile[:, b, no:no + ns], in_=pt[:, :ns])

            def agg(lyr, gidx, pre, gsel, psum_tag):
                """Aggregate one gate: returns list of 2 PSUM tiles [ns, BL, R, H].
                pre: 2 tiles [ms, BL, (G,) R, H]; gsel: gate index or None."""
                out = []
                for ic, (no, ns) in enumerate(NCH):
                    pg = aggps.tile([128, BL, R, H], F32, tag="agg", name="agg")
                    nc.tensor.matmul(pg[:ns], ones[:, no:no + ns],
                                     btile[0:1, lyr * 3 + gidx].rearrange("o b r h -> o (b r h)"),
                                     start=True, stop=False)
                    for q in range(R):
                        for im, (mo, ms) in enumerate(NCH):
                            last = (q == R - 1) and (im == len(NCH) - 1)
                            lt = adjts[q][im][:ms, no:no + ns]
                            src = pre[im][:ms] if gsel is None else pre[im][:ms, :, gsel]
                            if q == 0:
                                nc.tensor.matmul(pg[:ns], lt, src,
                                                 start=False, stop=last)
                            else:
                                nc.tensor.matmul(pg[:ns, :, q:R], lt, src[:, :, 0:R - q],
                                                 start=False, stop=False)
                                nc.tensor.matmul(pg[:ns, :, 0:q], lt, src[:, :, R - q:R],
                                                 start=False, stop=last)
                    out.append(pg)
                return out

            def cell(lyr, x_a, h_a, h_b, t_):
                """One TensorGRUCell; returns (hnew_a, hnew_b)."""
                # --- FT z,r gates ---
                pzr = [ppool.tile([128, BL, 2, R, H], F32, tag="pzr0", name="pzr0"),
                       ppool.tile([128, BL, 2, R, H], F32, tag="pzr1", name="pzr1")]
                for ic, (no, ns) in enumerate(NCH):
                    for b in range(BL):
                        ps1 = ftps.tile([128, 2, R, H], F32, tag="ftzr", name="ftzr")
                        nc.tensor.matmul(ps1[:ns], x_a[:, b, no:no + ns],
                                         wx[lyr][:, 0:2], start=True, stop=h_a is None)
                        if h_a is not None:
                            nc.tensor.matmul(ps1[:ns], h_a[:, b, no:no + ns],
                                             wh[lyr][:], start=False, stop=True)
                        nc.any.tensor_copy(out=pzr[ic][:ns, b], in_=ps1[:ns])

                # --- Agg z, r + sigmoid ---
                pz = agg(lyr, 0, pzr, 0, "az")
                pr = agg(lyr, 1, pzr, 1, "ar")
                zb, rb = [], []
                for ic, (no, ns) in enumerate(NCH):
                    zt = gpool.tile([128, BL, R, H], F32, tag="zb", name="zb")
                    rt = gpool.tile([128, BL, R, H], F32, tag="rb", name="rb")
                    nc.scalar.activation(out=zt[:ns], in_=pz[ic][:ns], func=SIG, scale=1.0)
                    nc.scalar.activation(out=rt[:ns], in_=pr[ic][:ns], func=SIG, scale=1.0)
                    zb.append(zt)
                    rb.append(rt)

                # --- Rg*Hs -> A layout ---
                rha = None
                if h_b is not None:
                    rhs_b = []
                    for ic, (no, ns) in enumerate(NCH):
                        rh = wpool.tile([128, BL, R, H], F32, tag="rghs", name="rghs")
                        eng = nc.vector if ic == 0 else nc.gpsimd
                        eng.tensor_tensor(rh[:ns], rb[ic][:ns], h_b[ic][:ns],
                                          mybir.AluOpType.mult)
                        rhs_b.append(rh)
                    rha = wpool.tile([R * H, BL, N], F32, tag="rha", name="rha")
                    transpose_to_a(rhs_b, rha)

                # --- FT h gate (X part + RgHs part in one PSUM) ---
                ph = [ppool.tile([128, BL, R, H], F32, tag="ph0", name="ph0"),
                      ppool.tile([128, BL, R, H], F32, tag="ph1", name="ph1")]
                for ic, (no, ns) in enumerate(NCH):
                    for b in range(BL):
                        ps2 = ftps.tile([128, R, H], F32, tag="fth", name="fth")
                        nc.tensor.matmul(ps2[:ns], x_a[:, b, no:no + ns],
                                         wx[lyr][:, 2], start=True, stop=rha is None)
                        if rha is not None:
                            nc.tensor.matmul(ps2[:ns], rha[:, b, no:no + ns],
                                             wr[lyr][:], start=False, stop=True)
                        nc.any.tensor_copy(out=ph[ic][:ns, b], in_=ps2[:ns])

                # --- Agg h + tanh ---
                pht = agg(lyr, 2, ph, None, "ah")
                htb = []
                for ic, (no, ns) in enumerate(NCH):
                    ht = gpool.tile([128, BL, R, H], F32, tag="htb", name="htb")
                    nc.scalar.activation(out=ht[:ns], in_=pht[ic][:ns], func=TANH, scale=1.0)
                    htb.append(ht)

                # --- mix: Hnew = Ht + Z*(Hs - Ht) ; t=0: Hnew = Ht - Z*Ht ---
                hnb = []
                for ic, (no, ns) in enumerate(NCH):
                    hn = hnbpool.tile([128, BL, R, H], F32, tag="hnb", name="hnb")
                    if h_b is None:
                        d = wpool.tile([128, BL, R, H], F32, tag="mixd", name="mixd")
                        nc.vector.tensor_tensor(d[:ns], zb[ic][:ns], htb[ic][:ns],
                                                mybir.AluOpType.mult)
                        nc.vector.tensor_tensor(hn[:ns], htb[ic][:ns], d[:ns],
                                                mybir.AluOpType.subtract)
                    else:
                        d = wpool.tile([128, BL, R, H], F32, tag="mixd", name="mixd")
                        nc.vector.tensor_tensor(d[:ns], h_b[ic][:ns], htb[ic][:ns],
                                                mybir.AluOpType.subtract)
                        eng = nc.gpsimd if ic == 0 else nc.vector
                        eng.tensor_tensor(d[:ns], d[:ns], zb[ic][:ns],
                                          mybir.AluOpType.mult)
                        nc.vector.tensor_tensor(hn[:ns], htb[ic][:ns], d[:ns],
                                                mybir.AluOpType.add)
                    hnb.append(hn)
                hna = hapool.tile([R * H, BL, N], F32, tag="hna", name="hna")
                transpose_to_a(hnb, hna)
                return hna, hnb

            # ---- recurrence ----
            hA = [None, None]
            hB = [None, None]
            for t_ in range(T):
                x1a = hA[0]  # layer1 consumes previous-step o0 for t>=1
                hA[0], hB[0] = cell(0, xa[t_], hA[0], hB[0], t_)
                if t_ == 0:
                    x1a = hA[0]
                hA[1], hB[1] = cell(1, x1a, hA[1], hB[1], t_)
                for ic, (no, ns) in enumerate(NCH):
                    nc.sync.dma_start(
                        out=outp[t_, no:no + ns, :],
                        in_=hB[1][ic][:ns].rearrange("p b r h -> p (b r h)"))
            for l in range(2):
                for ic, (no, ns) in enumerate(NCH):
                    nc.sync.dma_start(
                        out=hlast[l, no:no + ns, :],
                        in_=hB[l][ic][:ns].rearrange("p b r h -> p (b r h)"))
    nc.compile()
    return nc


def _get_nc():
    if "nc" not in _cache:
        nc = bacc.Bacc("TRN2", target_bir_lowering=False, debug=False,
                       num_devices=NCORES)
        _cache["nc"] = _emit(nc)
    return _cache["nc"]


def kernel(inputs, U, Wxz0, Wxr0, Wxh0, Whz0, Whr0, B0,
           Wxz1, Wxr1, Wxh1, Whz1, Whr1, B1):
    args = [np.asarray(a, np.float32) for a in
            (inputs, U, Wxz0, Wxr0, Wxh0, Whz0, Whr0, B0,
             Wxz1, Wxr1, Wxh1, Whz1, Whr1, B1)]
    adjt, wcx, wch, wcr, brow, xs = host_prep(*args)
    nc = _get_nc()
    shared = dict(adjt=adjt, wcx0=wcx[0], wcx1=wcx[1], wch0=wch[0],
                  wch1=wch[1], wcr0=wcr[0], wcr1=wcr[1], brow=brow)
    in_maps = [dict(shared, xin=xs[c]) for c in range(NCORES)]
    res = run_bass_kernel_spmd(nc, in_maps, list(range(NCORES)),
                               **_cache.get("run_kwargs", {}))
    _cache["last_results"] = res
    outs, hls = [], []
    for c in range(NCORES):
        o = res.results[c]["o"].reshape(T, N, BL, R, H)
        outs.append(np.ascontiguousarray(o.transpose(2, 0, 1, 4, 3)))
        hl = res.results[c]["hl"].reshape(2, N, BL, R, H)
        hls.append(np.ascontiguousarray(hl.transpose(0, 2, 1, 4, 3)))
    out = np.concatenate(outs, axis=0).astype(np.float32)
    h_last = np.concatenate(hls, axis=1).astype(np.float32)
    return out, h_last


# revision 10
# speedup vs baseline: 1.1242x; 1.0254x over previous
"""ATG-GRU (t-product graph GRU) Trainium2 kernel.

Sharding: data-parallel over batch B=32 across 8 cores (B_local=4).
Math: real-domain block-circulant formulation of the t-product gconv:
  gconv(X)[b,n,h,r] = sum_{m,f,r1+r2+r3=r (mod R)} adj[n,m,r1] X[b,m,f,r2] W[f,h,r3]
Two matmul stages per gate group:
  FT  (contract (r',f)):  lhsT = activation in "A" layout [(r',f), (b,n)],
                          rhs = host-built circulant weight Wc [(r',f),(g,r,h)]
                          -> PSUM [n, (g,r,h)] per (b, n-chunk)  ("B" layout)
  Agg (contract m, circular shift over r): lhsT = adjT_q [m, n'], rhs = pre-acts
      in B layout; accumulate over q with r-block rotation of the free axis;
      bias added via rank-1 matmul (ones row x bias row).
All matmuls full fp32 (the recurrence amplifies rounding ~2e3x; fp16/fp32r operands lose ~all precision).
"""
import sys

sys.path.insert(0, "/opt/trn_rl_repo")

import numpy as np

import concourse.bass as bass
import concourse.mybir as mybir
import concourse.tile as tile
from concourse import bacc
from concourse.bass_utils import run_bass_kernel_spmd
from concourse.masks import make_identity

B, T, N, F, H, R, E = 32, 8, 150, 16, 32, 4, 16
NCORES = 8
BL = B // NCORES          # 4 local batch
NCH = [(0, 128), (128, N - 128)]   # node chunks (offset, size)
F16 = mybir.dt.float16
F32 = mybir.dt.float32

_cache = {}


def host_prep(inputs, U, Wxz0, Wxr0, Wxh0, Whz0, Whr0, B0,
              Wxz1, Wxr1, Wxh1, Whz1, Whr1, B1):
    """Host-side layout transforms + the tiny adjacency computation."""
    Uf = np.fft.fft(U.astype(np.float64), axis=-1)
    A = np.fft.ifft(np.einsum('ner,mer->nmr', Uf, Uf), axis=-1).real
    A = np.maximum(A, 0.0)
    A = A - A.max(axis=1, keepdims=True)
    Ea = np.exp(A)
    adj = (Ea / Ea.sum(axis=1, keepdims=True)).astype(np.float32)  # [N,N,R]
    adjt = np.ascontiguousarray(adj.transpose(2, 1, 0))  # [q, m, n'] = adj[n',m,q]

    def circ(W):
        """W [D,H,R] -> Wc [(r'*D+f), (r*H+h)] = W[f,h,(r-r')%R]."""
        D = W.shape[0]
        Wc = np.zeros((R * D, R * H), np.float32)
        for rp in range(R):
            for r in range(R):
                Wc[rp * D:(rp + 1) * D, r * H:(r + 1) * H] = W[:, :, (r - rp) % R]
        return Wc

    wcx = [np.concatenate([circ(Wxz0), circ(Wxr0), circ(Wxh0)], axis=1),
           np.concatenate([circ(Wxz1), circ(Wxr1), circ(Wxh1)], axis=1)]
    wch = [np.concatenate([circ(Whz0), circ(Whr0)], axis=1),
           np.concatenate([circ(Whz1), circ(Whr1)], axis=1)]
    wcr = [circ(Whr0), circ(Whr1)]
    # bias rows: brow[(l,g), (b, ro, h)] = B_l[g, h, ro]
    brow = np.zeros((6, BL * R * H), np.float32)
    for l, Bb in ((0, B0), (1, B1)):
        for g in range(3):
            brow[l * 3 + g] = np.tile(Bb[g].T.reshape(-1), BL)
    brow = brow.reshape(1, 6 * BL * R * H)
    xs = []
    for c in range(NCORES):
        xl = inputs[c * BL:(c + 1) * BL]                    # [BL,T,N,F,R]
        xs.append(np.ascontiguousarray(
            xl.transpose(1, 4, 3, 0, 2).reshape(T, R * F, BL * N)).astype(np.float32))
    return adjt, wcx, wch, wcr, brow, xs


def _emit(nc):
    dp = nc.declare_dram_parameter
    xin = dp("xin", [T, R * F, BL * N], F32, isOutput=False)
    adjt = dp("adjt", [R, N, N], F32, isOutput=False)
    wcx0 = dp("wcx0", [R * F, 3 * R * H], F32, isOutput=False)
    wcx1 = dp("wcx1", [R * H, 3 * R * H], F32, isOutput=False)
    wch0 = dp("wch0", [R * H, 2 * R * H], F32, isOutput=False)
    wch1 = dp("wch1", [R * H, 2 * R * H], F32, isOutput=False)
    wcr0 = dp("wcr0", [R * H, R * H], F32, isOutput=False)
    wcr1 = dp("wcr1", [R * H, R * H], F32, isOutput=False)
    brow = dp("brow", [1, 6 * BL * R * H], F32, isOutput=False)
    outp = dp("o", [T, N, BL * R * H], F32, isOutput=True)
    hlast = dp("hl", [2, N, BL * R * H], F32, isOutput=True)

    SIG = mybir.ActivationFunctionType.Sigmoid
    TANH = mybir.ActivationFunctionType.Tanh

    with tile.TileContext(nc) as tc:
        with tc.tile_pool(name="const", bufs=1) as cpool, \
             tc.tile_pool(name="xa", bufs=1) as xpool, \
             tc.tile_pool(name="ha", bufs=5) as hapool, \
             tc.tile_pool(name="hnb", bufs=10) as hnbpool, \
             tc.tile_pool(name="gat", bufs=4) as gpool, \
             tc.tile_pool(name="work", bufs=4) as wpool, \
             tc.tile_pool(name="pre", bufs=3) as ppool, \
             tc.tile_pool(name="ftps", bufs=2, space="PSUM") as ftps, \
             tc.tile_pool(name="aggps", bufs=3, space="PSUM") as aggps, \
             tc.tile_pool(name="trps", bufs=1, space="PSUM") as trps:

            # ---- constants ----
            wx = [cpool.tile([R * F, 3, R, H], F32, tag="wcx0", name="wcx0"),
                  cpool.tile([R * H, 3, R, H], F32, tag="wcx1", name="wcx1")]
            nc.sync.dma_start(out=wx[0][:], in_=wcx0.ap().rearrange("k (g r h) -> k g r h", g=3, r=R))
            nc.sync.dma_start(out=wx[1][:], in_=wcx1.ap().rearrange("k (g r h) -> k g r h", g=3, r=R))
            wh = [cpool.tile([R * H, 2, R, H], F32, tag="wch0", name="wch0"),
                  cpool.tile([R * H, 2, R, H], F32, tag="wch1", name="wch1")]
            nc.sync.dma_start(out=wh[0][:], in_=wch0.ap().rearrange("k (g r h) -> k g r h", g=2, r=R))
            nc.sync.dma_start(out=wh[1][:], in_=wch1.ap().rearrange("k (g r h) -> k g r h", g=2, r=R))
            wr = [cpool.tile([R * H, R, H], F32, tag="wcr0", name="wcr0"),
                  cpool.tile([R * H, R, H], F32, tag="wcr1", name="wcr1")]
            nc.sync.dma_start(out=wr[0][:], in_=wcr0.ap().rearrange("k (r h) -> k r h", r=R))
            nc.sync.dma_start(out=wr[1][:], in_=wcr1.ap().rearrange("k (r h) -> k r h", r=R))
            adjts = []
            for q in range(R):
                row = []
                for (mo, ms) in NCH:
                    t_ = cpool.tile([128, N], F32, tag=f"adjt{q}{mo}", name=f"adjt{q}{mo}")
                    nc.sync.dma_start(out=t_[:ms], in_=adjt[q, mo:mo + ms, :])
                    row.append(t_)
                adjts.append(row)
            btile = cpool.tile([1, 6, BL, R, H], F32, tag="brow", name="brow")
            nc.sync.dma_start(out=btile[:], in_=brow.ap().rearrange(
                "o (s b r h) -> o s b r h", s=6, b=BL, r=R))
            ones = cpool.tile([1, N], F32, tag="ones", name="ones")
            nc.vector.memset(ones[:], 1.0)
            ident = cpool.tile([128, 128], F32, tag="ident", name="ident")
            make_identity(nc, ident[:])
            xa = []
            for t_ in range(T):
                xt = xpool.tile([R * F, BL, N], F32, tag=f"x{t_}", name=f"x{t_}")
                nc.sync.dma_start(out=xt[:], in_=xin[t_].rearrange("k (b n) -> k b n", b=BL))
                xa.append(xt)

            def transpose_to_a(src_b, dtile):
                """src_b: 2 B-layout fp32 tiles [ns, BL, R, H] -> dtile fp32 [128, BL, N]."""
                for b in range(BL):
                    for ic, (no, ns) in enumerate(NCH):
                        pt = trps.tile([128, 128], F32, tag="trp", name="trp")
                        src = src_b[ic][:ns, b].rearrange("p r h -> p (r h)")
                        nc.tensor.transpose(pt[:, :ns], src, ident[:ns, :ns])
                        nc.any.tensor_copy(out=dtile[:, b, no:no + ns], in_=pt[:, :ns])

            def agg(lyr, gidx, pre, gsel, psum_tag):
                """Aggregate one gate: returns list of 2 PSUM tiles [ns, BL, R, H].
                pre: 2 tiles [ms, BL, (G,) R, H]; gsel: gate index or None."""
                out = []
                for ic, (no, ns) in enumerate(NCH):
                    pg = aggps.tile([128, BL, R, H], F32, tag="agg", name="agg")
                    nc.tensor.matmul(pg[:ns], ones[:, no:no + ns],
                                     btile[0:1, lyr * 3 + gidx].rearrange("o b r h -> o (b r h)"),
                                     start=True, stop=False)
                    for q in range(R):
                        for im, (mo, ms) in enumerate(NCH):
                            last = (q == R - 1) and (im == len(NCH) - 1)
                            lt = adjts[q][im][:ms, no:no + ns]
                            src = pre[im][:ms] if gsel is None else pre[im][:ms, :, gsel]
                            if q == 0:
                                nc.tensor.matmul(pg[:ns], lt, src,
                                                 start=False, stop=last)
                            else:
                                nc.tensor.matmul(pg[:ns, :, q:R], lt, src[:, :, 0:R - q],
                                                 start=False, stop=False)
                                nc.tensor.matmul(pg[:ns, :, 0:q], lt, src[:, :, R - q:R],
                                                 start=False, stop=last)
                    out.append(pg)
                return out

            def cell(lyr, x_a, h_a, h_b, t_):
                """One TensorGRUCell; returns (hnew_a, hnew_b)."""
                # --- FT z,r gates ---
                pzr = [ppool.tile([128, BL, 2, R, H], F32, tag="pzr0", name="pzr0"),
                       ppool.tile([128, BL, 2, R, H], F32, tag="pzr1", name="pzr1")]
                for ic, (no, ns) in enumerate(NCH):
                    for bp in range(BL // 2):
                        ps1 = ftps.tile([128, 2, 2, R, H], F32, tag="ftzr", name="ftzr")
                        for b2 in range(2):
                            b = bp * 2 + b2
                            nc.tensor.matmul(ps1[:ns, b2], x_a[:, b, no:no + ns],
                                             wx[lyr][:, 0:2], start=True, stop=h_a is None)
                            if h_a is not None:
                                nc.tensor.matmul(ps1[:ns, b2], h_a[:, b, no:no + ns],
                                                 wh[lyr][:], start=False, stop=True)
                        nc.any.tensor_copy(out=pzr[ic][:ns, bp * 2:bp * 2 + 2],
                                           in_=ps1[:ns])

                # --- Agg z, r + sigmoid ---
                pz = agg(lyr, 0, pzr, 0, "az")
                pr = agg(lyr, 1, pzr, 1, "ar")
                zb, rb = [], []
                for ic, (no, ns) in enumerate(NCH):
                    zt = gpool.tile([128, BL, R, H], F32, tag="zb", name="zb")
                    rt = gpool.tile([128, BL, R, H], F32, tag="rb", name="rb")
                    nc.scalar.activation(out=zt[:ns], in_=pz[ic][:ns], func=SIG, scale=1.0)
                    nc.scalar.activation(out=rt[:ns], in_=pr[ic][:ns], func=SIG, scale=1.0)
                    zb.append(zt)
                    rb.append(rt)

                # --- Rg*Hs -> A layout ---
                rha = None
                if h_b is not None:
                    rhs_b = []
                    for ic, (no, ns) in enumerate(NCH):
                        rh = wpool.tile([128, BL, R, H], F32, tag="rghs", name="rghs")
                        eng = nc.vector if ic == 0 else nc.gpsimd
                        eng.tensor_tensor(rh[:ns], rb[ic][:ns], h_b[ic][:ns],
                                          mybir.AluOpType.mult)
                        rhs_b.append(rh)
                    rha = wpool.tile([R * H, BL, N], F32, tag="rha", name="rha")
                    transpose_to_a(rhs_b, rha)

                # --- FT h gate (X part + RgHs part in one PSUM) ---
                ph = [ppool.tile([128, BL, R, H], F32, tag="ph0", name="ph0"),
                      ppool.tile([128, BL, R, H], F32, tag="ph1", name="ph1")]
                for ic, (no, ns) in enumerate(NCH):
                    ps2 = ftps.tile([128, BL, R, H], F32, tag="fth", name="fth")
                    for b in range(BL):
                        nc.tensor.matmul(ps2[:ns, b], x_a[:, b, no:no + ns],
                                         wx[lyr][:, 2], start=True, stop=rha is None)
                        if rha is not None:
                            nc.tensor.matmul(ps2[:ns, b], rha[:, b, no:no + ns],
                                             wr[lyr][:], start=False, stop=True)
                    nc.any.tensor_copy(out=ph[ic][:ns], in_=ps2[:ns])

                # --- Agg h + tanh ---
                pht = agg(lyr, 2, ph, None, "ah")
                htb = []
                for ic, (no, ns) in enumerate(NCH):
                    ht = gpool.tile([128, BL, R, H], F32, tag="htb", name="htb")
                    nc.scalar.activation(out=ht[:ns], in_=pht[ic][:ns], func=TANH, scale=1.0)
                    htb.append(ht)

                # --- mix: Hnew = Ht + Z*(Hs - Ht) ; t=0: Hnew = Ht - Z*Ht ---
                hnb = []
                for ic, (no, ns) in enumerate(NCH):
                    hn = hnbpool.tile([128, BL, R, H], F32, tag="hnb", name="hnb")
                    if h_b is None:
                        d = wpool.tile([128, BL, R, H], F32, tag="mixd", name="mixd")
                        nc.vector.tensor_tensor(d[:ns], zb[ic][:ns], htb[ic][:ns],
                                                mybir.AluOpType.mult)
                        nc.vector.tensor_tensor(hn[:ns], htb[ic][:ns], d[:ns],
                                                mybir.AluOpType.subtract)
                    else:
                        d = wpool.tile([128, BL, R, H], F32, tag="mixd", name="mixd")
                        nc.vector.tensor_tensor(d[:ns], h_b[ic][:ns], htb[ic][:ns],
                                                mybir.AluOpType.subtract)
                        eng = nc.gpsimd if ic == 0 else nc.vector
                        eng.tensor_tensor(d[:ns], d[:ns], zb[ic][:ns],
                                          mybir.AluOpType.mult)
                        nc.vector.tensor_tensor(hn[:ns], htb[ic][:ns], d[:ns],
                                                mybir.AluOpType.add)
                    hnb.append(hn)
                hna = hapool.tile([R * H, BL, N], F32, tag="hna", name="hna")
                transpose_to_a(hnb, hna)
                return hna, hnb

            # ---- recurrence ----
            hA = [None, None]
            hB = [None, None]
            for t_ in range(T):
                x1a = hA[0]  # layer1 consumes previous-step o0 for t>=1
                hA[0], hB[0] = cell(0, xa[t_], hA[0], hB[0], t_)
                if t_ == 0:
                    x1a = hA[0]
                hA[1], hB[1] = cell(1, x1a, hA[1], hB[1], t_)
                for ic, (no, ns) in enumerate(NCH):
                    nc.sync.dma_start(
                        out=outp[t_, no:no + ns, :],
                        in_=hB[1][ic][:ns].rearrange("p b r h -> p (b r h)"))
            for l in range(2):
                for ic, (no, ns) in enumerate(NCH):
                    nc.sync.dma_start(
                        out=hlast[l, no:no + ns, :],
                        in_=hB[l][ic][:ns].rearrange("p b r h -> p (b r h)"))
    nc.compile()
    return nc


def _get_nc():
    if "nc" not in _cache:
        nc = bacc.Bacc("TRN2", target_bir_lowering=False, debug=False,
                       num_devices=NCORES)
        _cache["nc"] = _emit(nc)
    return _cache["nc"]


def kernel(inputs, U, Wxz0, Wxr0, Wxh0, Whz0, Whr0, B0,
           Wxz1, Wxr1, Wxh1, Whz1, Whr1, B1):
    args = [np.asarray(a, np.float32) for a in
            (inputs, U, Wxz0, Wxr0, Wxh0, Whz0, Whr0, B0,
             Wxz1, Wxr1, Wxh1, Whz1, Whr1, B1)]
    adjt, wcx, wch, wcr, brow, xs = host_prep(*args)
    nc = _get_nc()
    shared = dict(adjt=adjt, wcx0=wcx[0], wcx1=wcx[1], wch0=wch[0],
                  wch1=wch[1], wcr0=wcr[0], wcr1=wcr[1], brow=brow)
    in_maps = [dict(shared, xin=xs[c]) for c in range(NCORES)]
    res = run_bass_kernel_spmd(nc, in_maps, list(range(NCORES)),
                               **_cache.get("run_kwargs", {}))
    _cache["last_results"] = res
    outs, hls = [], []
    for c in range(NCORES):
        o = res.results[c]["o"].reshape(T, N, BL, R, H)
        outs.append(np.ascontiguousarray(o.transpose(2, 0, 1, 4, 3)))
        hl = res.results[c]["hl"].reshape(2, N, BL, R, H)
        hls.append(np.ascontiguousarray(hl.transpose(0, 2, 1, 4, 3)))
    out = np.concatenate(outs, axis=0).astype(np.float32)
    h_last = np.concatenate(hls, axis=1).astype(np.float32)
    return out, h_last
